# revision 1
# baseline (speedup 1.0000x reference)
"""Trainium2 Bass kernel for nn_Cheby_KAN (FourierFeatures -> 5 Cheby-KAN layers -> cheby-gelu).

Self-contained: hardcodes shapes/sharding. Data-parallel over 8 NeuronCores
(batch 131072 -> 16384 rows/core).

Key design (see git-less history in comments):
  - Each KAN layer out[q,b] = sum_{p,n} tW[q,p,n] W[q,p] U_n(tanh(h[p,b]))
    (U_n = Chebyshev 2nd kind; the reference's T~ recurrence with T~_1=2x).
    We evaluate it as sum_{p,k} C[p,k,q] phi_k(xt) where phi is a
    Chebyshev-conditioned product basis built in 1 op/tile:
      phi0=1, phi1=x, phi2=2x^2(ACT Square scale=sqrt2)|x^2(DVE mult),
      phi3=(phi2-1)*phi1 (DVE STT), phi4=Square(a*phi2+b)=T4+1 (ACT),
      phi5=(phi4-1)*phi1, phi6=phi3^2 (ACT|DVE), phi7=(phi4-1)*phi3,
      phi8=Square(s2*phi4-s2)=T8+1 (ACT)
    The exact transform D (U->phi) is solved numerically on host per layer
    variant and folded into the weights. The bounded transform keeps
    rounding amplification small (raw monomials would be ~100x worse).
  - Matmuls run in fp32 (all L_F32R False). float32r would stream 4x faster
    on the PE (fp32 is lowered to 2 half-rate passes) but its ~12-bit
    truncating operand rounding, amplified ~5-40x by this network's
    residual Jacobian and the sum's cancellation factor, measured 2e-3..2e-2
    output error - too risky against an fp32-envelope accuracy gate. The
    L_F32R knob per layer remains for experimentation.
  - Residuals accumulate in PSUM across layers 1-3 (start=False): the
    residual adds and all evacuations cost zero pointwise ops.
  - L0 contraction K=64: tiles are computed partition-duplicated [128,G]
    (free: pointwise cost is free-dim-bound).
  - Fourier stage in "turns" with exact DVE range reduction; one Sin op
    computes sin and cos via a shift row folded into the proj matmul.
  - Final cheby-gelu (standard T_n, gelu exact) on [128,128] after a DRAM
    round-trip reshape of h5.
"""

import math

import numpy as np

NCORES = 8
BATCH = 131072
R = BATCH // NCORES  # rows per core
G = 2048  # free-dim group size
F = 512  # matmul moving chunk (fp32 psum bank)
NG = R // G
NCH = G // F
DEG = 8

GELU_1 = 0.8413447460685429  # gelu(1), exact
S2 = float(np.float32(math.sqrt(2.0)))
S22 = float(np.float32(2.0 * math.sqrt(2.0)))
RC = 12582912.0  # 1.5 * 2**23 fp32 round-to-int constant

# per-layer engine variants for the flexible tiles (ACT if True else DVE)
PHI2_ACT = [True, True, True, True, True]
PHI6_ACT = [False, True, True, False, False]
# per-layer matmul dtype: True -> float32r (4x faster PE, ~12-bit truncating
# operands), False -> float32. Early layers' noise is amplified ~5-40x
# downstream, so keep them fp32.
L_F32R = [False, False, False, False, False]
# pair L0's K=64 chunks into PE row groups (runtime-broken on this HW: off)
L0_PAIR = False


def _pmul(a, b):
    return np.convolve(a, b)[:9]


def _phi_polys(phi2_act, phi6_act):
    """Polynomial (power-coeff) vectors of the 9 phi tiles for a variant."""
    x = np.zeros(9)
    x[1] = 1.0
    one = np.zeros(9)
    one[0] = 1.0
    p = [None] * 9
    p[0] = one
    p[1] = x
    if phi2_act:
        p[2] = _pmul(S2 * x, S2 * x)  # Square(s2*xt) = 2x^2
        a4, b4 = S2, -S2  # (s2*phi2 - s2)^2 = 2(2x^2-1)^2
    else:
        p[2] = _pmul(x, x)  # xt*xt
        a4, b4 = S22, -S2  # (2s2*x^2 - s2)^2 = 2(2x^2-1)^2
    p[3] = _pmul(p[2] - one, p[1])
    p[4] = _pmul(a4 * p[2] + b4 * one, a4 * p[2] + b4 * one)
    p[5] = _pmul(p[4] - one, p[1])
    p[6] = _pmul(p[3], p[3])
    p[7] = _pmul(p[4] - one, p[3])
    p[8] = _pmul(S2 * p[4] - S2 * one, S2 * p[4] - S2 * one)
    return np.stack(p, 0), (a4, b4)


def _u_mono():
    M = np.zeros((9, 9))
    M[0, 0] = 1.0
    M[1, 1] = 2.0
    for n in range(2, 9):
        M[n, 1:] += 2.0 * M[n - 1, :-1]
        M[n, :] -= M[n - 2, :]
    return M


def _fold_layer(W, tW, l):
    """C[p,k,q] (float32, contiguous) for layer l's phi variant."""
    PHI, _ = _phi_polys(PHI2_ACT[l], PHI6_ACT[l])
    D = _u_mono() @ np.linalg.inv(PHI)  # U_n = sum_k D[n,k] phi_k
    A = tW.astype(np.float64) * W.astype(np.float64)[:, :, None]  # [q,p,n]
    C = np.einsum("qpn,nk->pkq", A, D)
    return np.ascontiguousarray(C, dtype=np.float32)


def _pack_c0(C0):
    """[64,9,128] -> [128,5,128]: slot 0 = k0 (top only); slots 1-4 stack the
    pairs (1,2),(3,4),(6,5),(7,8) along the contraction partitions."""
    c0p = np.zeros((128, 5, 128), dtype=np.float32)
    c0p[0:64, 0, :] = C0[:, 0, :]
    for j, (kt, kb) in enumerate([(1, 2), (3, 4), (6, 5), (7, 8)], start=1):
        c0p[0:64, j, :] = C0[:, kt, :]
        c0p[64:128, j, :] = C0[:, kb, :]
    return np.ascontiguousarray(c0p)


_CACHE = {}

TRACE = False
TRACE_KWARGS = {}
LAST_RESULTS = None


def _build():
    from concourse import bacc, bass, tile
    import concourse.mybir as mybir
    from concourse._compat import get_trn_type

    A = mybir.ActivationFunctionType
    OP = mybir.AluOpType
    f32 = mybir.dt.float32
    f32r = mybir.dt.float32r

    nc = bacc.Bacc(
        get_trn_type() or "TRN2",
        target_bir_lowering=False,
        debug=False,
        num_devices=NCORES,
    )

    # ---- DRAM I/O ----
    # xT row 8 is all-ones; b2 row 8 is the sin/cos shift so the proj matmul
    # yields m + shift (turns), duplicated twice over 128 partitions.
    xT_d = nc.dram_tensor("xT", [9, R], f32, kind="ExternalInput").ap()
    b2_d = nc.dram_tensor("b2", [9, 128], f32, kind="ExternalInput").ap()
    c_d = [
        nc.dram_tensor("c0", [128, 5, 128], f32, kind="ExternalInput").ap(),  # paired
        nc.dram_tensor("c1", [128, 9, 128], f32, kind="ExternalInput").ap(),
        nc.dram_tensor("c2", [128, 9, 128], f32, kind="ExternalInput").ap(),
        nc.dram_tensor("c3", [128, 9, 128], f32, kind="ExternalInput").ap(),
        nc.dram_tensor("c4", [128, 9, 1], f32, kind="ExternalInput").ap(),
    ]
    bv_d = nc.dram_tensor("biasv", [128, 5], f32, kind="ExternalInput").ap()
    h5_d = nc.dram_tensor("h5scratch", [R], f32, kind="Internal").ap()
    out_d = nc.dram_tensor("out", [128, R // 128], f32, kind="ExternalOutput").ap()

    with tile.TileContext(nc) as tc:
        with (
            tc.tile_pool(name="wpool", bufs=1) as wpool,
            tc.tile_pool(name="const", bufs=1) as cpool,
            tc.tile_pool(name="xpool", bufs=2) as xpool,
            tc.tile_pool(name="h0pool", bufs=2) as h0pool,
            tc.tile_pool(name="bpool", bufs=2) as bpool,
            tc.tile_pool(name="b1pool", bufs=1) as b1pool,
            tc.tile_pool(name="h5pool", bufs=2) as h5pool,
            tc.tile_pool(name="fpool", bufs=1) as fpool,
            tc.tile_pool(name="hps_pool", bufs=1, space=bass.MemorySpace.PSUM) as hpsp,
            tc.tile_pool(name="proj_pool", bufs=2, space=bass.MemorySpace.PSUM) as prjp,
            tc.tile_pool(name="l4_pool", bufs=2, space=bass.MemorySpace.PSUM) as l4p,
        ):
            # ---- constants / weights ----
            b2_sb = wpool.tile([9, 128], f32)
            nc.sync.dma_start(out=b2_sb[:], in_=b2_d)
            c_sb = []
            for l, cd in enumerate(c_d):
                q = 1 if l == 4 else 128
                nk = 5 if l == 0 else 9
                if L_F32R[l]:
                    stage = wpool.tile([128, 9, 128], f32, tag="cstage")
                    nc.sync.dma_start(out=stage[:, :, :q], in_=cd)
                    tr = wpool.tile([128, 9, q], f32r, tag=f"c{l}r")
                    nc.vector.tensor_copy(tr[:], stage[:, :, :q])
                    c_sb.append(tr)
                else:
                    t32 = wpool.tile([128, nk, q], f32, tag=f"c{l}")
                    nc.sync.dma_start(out=t32[:], in_=cd)
                    c_sb.append(t32)

            ones = cpool.tile([128, F], f32)
            nc.vector.memset(ones[:], 1.0)
            ones_r = cpool.tile([128, F], f32r)
            nc.vector.tensor_copy(ones_r[:], ones[:])
            bS2 = cpool.tile([128, 1], f32)
            nc.vector.memset(bS2[:], -S2)
            ones1 = cpool.tile([128, 1], f32)
            nc.vector.memset(ones1[:], 1.0)

            def powers(xt32, l, p, dt):
                """phi_1..phi_8 tiles ([p, G]); index 0 = ones.

                xt32 is the fp32 tanh tile. The phi2/phi4 ladder ingredients
                stay fp32 (f32r rounding would compound ~16x into phi8);
                their f32r rhs copies are cast on the otherwise-idle GpSimd.
                """
                x = [None] * 9
                x[1] = xt32  # dt-typed tanh output (rhs + STT ingredient)
                x2 = bpool.tile([p, G], dt, tag="pw2")
                if PHI2_ACT[l]:
                    nc.scalar.activation(x2[:], xt32[:], A.Square, scale=S2)
                    a4 = S2
                else:
                    nc.vector.tensor_tensor(x2[:], xt32[:], xt32[:], op=OP.mult)
                    a4 = S22
                x[2] = x2
                x3 = bpool.tile([p, G], dt, tag="pw3")
                nc.vector.scalar_tensor_tensor(
                    x3[:], x2[:], -1.0, xt32[:], op0=OP.add, op1=OP.mult
                )
                x4 = bpool.tile([p, G], dt, tag="pw4")
                nc.scalar.activation(x4[:], x2[:], A.Square, bias=bS2[:], scale=a4)
                x[4] = x4
                x5 = bpool.tile([p, G], dt, tag="pw5")
                nc.vector.scalar_tensor_tensor(
                    x5[:], x4[:], -1.0, xt32[:], op0=OP.add, op1=OP.mult
                )
                x6 = bpool.tile([p, G], dt, tag="pw6")
                if PHI6_ACT[l]:
                    nc.scalar.activation(x6[:], x3[:], A.Square)
                else:
                    nc.vector.tensor_tensor(x6[:], x3[:], x3[:], op=OP.mult)
                x7 = bpool.tile([p, G], dt, tag="pw7")
                nc.vector.scalar_tensor_tensor(
                    x7[:], x4[:], -1.0, x3[:], op0=OP.add, op1=OP.mult
                )
                x8 = bpool.tile([p, G], dt, tag="pw8")
                nc.scalar.activation(x8[:], x4[:], A.Square, bias=bS2[:], scale=S2)
                x[3], x[5], x[6], x[7], x[8] = x3, x5, x6, x7, x8
                return x

            for g in range(NG):
                # Fourier: m+shift duplicated over 128 partitions; range-reduce; Sin.
                h0 = h0pool.tile([128, G], f32, tag="h0")
                for c in range(NCH):
                    xt_g = xpool.tile([9, F], f32, tag="xg")
                    nc.sync.dma_start(
                        out=xt_g[:], in_=xT_d[:, g * G + c * F : g * G + (c + 1) * F]
                    )
                    pr = prjp.tile([128, F], f32, tag="proj")
                    nc.tensor.matmul(pr[:], b2_sb[:], xt_g[:], start=True, stop=True)
                    rr = xpool.tile([128, F], f32, tag="rr")
                    nc.vector.tensor_scalar(
                        rr[:], pr[:], RC, RC, op0=OP.add, op1=OP.subtract
                    )
                    # frac = pr - rr, written over rr (lane-aligned in-place)
                    nc.vector.tensor_tensor(rr[:], pr[:], rr[:], op=OP.subtract)
                    nc.scalar.activation(
                        h0[:, c * F : (c + 1) * F], rr[:], A.Sin, scale=2.0 * math.pi
                    )

                hps = hpsp.tile([128, G], f32, tag="hps")

                # ---- layer 0: K=64 phi pairs stacked into K=128 matmuls.
                # Ingredients (xt, phi2, phi4) are partition-duplicated, so the
                # rhs pair tiles (phi1|phi2), (phi3|phi4), (phi6|phi5),
                # (phi7|phi8) are built lane-locally; 36 MMs/group become 20.
                xt_t = bpool.tile([128, G], f32, tag="pw1")
                nc.scalar.activation(xt_t[:], h0[:], A.Tanh)
                r12 = bpool.tile([128, G], f32, tag="pw3")
                nc.scalar.activation(r12[0:64, :], h0[0:64, :], A.Tanh)
                nc.scalar.activation(
                    r12[64:128, :], xt_t[64:128, :], A.Square, scale=S2
                )
                x2 = bpool.tile([128, G], f32, tag="pw2")
                nc.scalar.activation(x2[:], xt_t[:], A.Square, scale=S2)
                r34 = bpool.tile([128, G], f32, tag="pw5")
                nc.vector.scalar_tensor_tensor(
                    r34[0:64, :], x2[0:64, :], -1.0, xt_t[0:64, :],
                    op0=OP.add, op1=OP.mult,
                )
                nc.scalar.activation(
                    r34[64:128, :], x2[64:128, :], A.Square,
                    bias=bS2[64:128, :], scale=S2,
                )
                x4 = bpool.tile([128, G], f32, tag="pw4")
                nc.scalar.activation(x4[:], x2[:], A.Square, bias=bS2[:], scale=S2)
                r56 = bpool.tile([128, G], f32, tag="pw6")
                nc.vector.tensor_tensor(
                    r56[0:64, :], r34[0:64, :], r34[0:64, :], op=OP.mult
                )
                nc.vector.scalar_tensor_tensor(
                    r56[64:128, :], x4[64:128, :], -1.0, xt_t[64:128, :],
                    op0=OP.add, op1=OP.mult,
                )
                r78 = bpool.tile([128, G], f32, tag="pw7")
                nc.vector.scalar_tensor_tensor(
                    r78[0:64, :], x4[0:64, :], -1.0, r34[0:64, :],
                    op0=OP.add, op1=OP.mult,
                )
                nc.scalar.activation(
                    r78[64:128, :], x4[64:128, :], A.Square,
                    bias=bS2[64:128, :], scale=S2,
                )
                for c in range(NCH):
                    cf = slice(c * F, (c + 1) * F)
                    nc.tensor.matmul(
                        hps[:, cf],
                        c_sb[0][0:64, 0, :],
                        ones[0:64, :],
                        start=True,
                        stop=False,
                        skip_group_check=True,
                    )
                    for j, rt in enumerate((r12, r34, r56, r78), start=1):
                        nc.tensor.matmul(
                            hps[:, cf],
                            c_sb[0][:, j, :],
                            rt[:, cf],
                            start=False,
                            stop=False,
                            skip_group_check=True,
                        )

                # ---- layers 1..3: accumulate into hps ----
                for l in range(1, 4):
                    dt_l = f32r if L_F32R[l] else f32
                    ones_l = ones_r if L_F32R[l] else ones
                    xt_t = bpool.tile([128, G], dt_l, tag="pw1")
                    nc.scalar.activation(xt_t[:], hps[:], A.Tanh)
                    x = powers(xt_t, l, 128, dt_l)
                    for k in range(9):
                        for c in range(NCH):
                            cf = slice(c * F, (c + 1) * F)
                            rhs = ones_l[:] if k == 0 else x[k][:, cf]
                            nc.tensor.matmul(
                                hps[:, cf],
                                c_sb[l][:, k, :],
                                rhs,
                                start=False,
                                stop=(l == 3 and k == 8),
                                skip_group_check=True,
                            )

                # ---- layer 4 (q=1) ----
                dt_l = f32r if L_F32R[4] else f32
                ones_l = ones_r if L_F32R[4] else ones
                xt_t = bpool.tile([128, G], dt_l, tag="pw1")
                nc.scalar.activation(xt_t[:], hps[:], A.Tanh)
                x = powers(xt_t, 4, 128, dt_l)
                # q=1: contract k on the DVE (per-partition scalars from C4,
                # bias folded into the first op), then one K=128 ones-vector
                # matmul per chunk sums over p. Same fp32 math, 36->4 MMs/group.
                for c in range(NCH):
                    cf = slice(c * F, (c + 1) * F)
                    comb = b1pool.tile([128, F], f32, tag="comb")
                    nc.vector.tensor_scalar(
                        comb[:],
                        x[1][:, cf],
                        c_sb[4][:, 1, :],
                        c_sb[4][:, 0, :],
                        op0=OP.mult,
                        op1=OP.add,
                    )
                    for k in range(2, 9):
                        nc.vector.scalar_tensor_tensor(
                            comb[:],
                            x[k][:, cf],
                            c_sb[4][:, k, :],
                            comb[:],
                            op0=OP.mult,
                            op1=OP.add,
                        )
                    l4 = l4p.tile([1, F], f32, tag="l4")
                    nc.tensor.matmul(
                        l4[:], ones1[:], comb[:],
                        start=True, stop=True, skip_group_check=True,
                    )
                    h5c = h5pool.tile([1, F], f32, tag="h5c")
                    nc.scalar.activation(h5c[:], l4[:], A.Copy)
                    off = g * G + c * F
                    nc.sync.dma_start(out=h5_d[off : off + F], in_=h5c[:])

            # ---- final stage: cheby-gelu (standard T_n, deg 5) ----
            W = R // 128
            h5r = fpool.tile([128, W], f32, tag="h5r")
            nc.sync.dma_start(out=h5r[:], in_=h5_d.rearrange("(a b) -> a b", a=128))
            xt = fpool.tile([128, W], f32, tag="fxt")
            nc.scalar.activation(xt[:], h5r[:], A.Tanh)
            s = fpool.tile([128, W], f32, tag="fs")
            nc.scalar.activation(s[:], xt[:], A.Square)
            t2 = fpool.tile([128, W], f32, tag="ft2")
            nc.vector.tensor_scalar(t2[:], s[:], 2.0, -1.0, op0=OP.mult, op1=OP.add)
            t3 = fpool.tile([128, W], f32, tag="ft3")
            m = fpool.tile([128, W], f32, tag="fm")
            nc.vector.scalar_tensor_tensor(m[:], xt[:], 2.0, t2[:], op0=OP.mult, op1=OP.mult)
            nc.vector.tensor_tensor(t3[:], m[:], xt[:], op=OP.subtract)
            t4 = fpool.tile([128, W], f32, tag="ft4")
            m = fpool.tile([128, W], f32, tag="fm")
            nc.vector.scalar_tensor_tensor(m[:], xt[:], 2.0, t3[:], op0=OP.mult, op1=OP.mult)
            nc.vector.tensor_tensor(t4[:], m[:], t2[:], op=OP.subtract)
            t5 = fpool.tile([128, W], f32, tag="ft5")
            m = fpool.tile([128, W], f32, tag="fm")
            nc.vector.scalar_tensor_tensor(m[:], xt[:], 2.0, t4[:], op0=OP.mult, op1=OP.mult)
            nc.vector.tensor_tensor(t5[:], m[:], t3[:], op=OP.subtract)
            gsum = fpool.tile([128, W], f32, tag="fgsum")
            gt = fpool.tile([128, W], f32, tag="fgt")
            nc.scalar.activation(gsum[:], xt[:], A.Gelu)  # gelu(T1)
            for tt in (t2, t3, t4):
                nc.scalar.activation(gt[:], tt[:], A.Gelu)
                nc.vector.tensor_tensor(gsum[:], gsum[:], gt[:], op=OP.add)
            nc.scalar.activation(gt[:], t5[:], A.Gelu)
            nc.vector.scalar_tensor_tensor(
                s[:], gt[:], GELU_1, gsum[:], op0=OP.add, op1=OP.add
            )
            nc.sync.dma_start(out=out_d, in_=s[:])

    nc.compile()
    return nc


def _prepare_inputs(x, B, Ws, tWs):
    # turns units; 4 stacked copies of B (128 partitions of duplicated m+shift)
    b1 = np.concatenate([B, B, B, B], axis=1).astype(np.float32)  # [8, 128]
    qshift = np.concatenate([np.zeros(32), np.full(32, 0.25)])
    shift_row = np.concatenate([qshift, qshift]).astype(np.float32)  # [128]
    b2 = np.ascontiguousarray(np.vstack([b1, shift_row[None, :]]), dtype=np.float32)
    cs = [_fold_layer(W, tW, l) for l, (W, tW) in enumerate(zip(Ws, tWs))]
    biasv = np.zeros((128, 5), dtype=np.float32)
    for l in range(5):
        bl = cs[l][:, 0, :].astype(np.float64).sum(axis=0)  # ones-tile contribution
        biasv[: bl.shape[0], l] = bl.astype(np.float32)
    shared = {
        "biasv": biasv,
        "b2": b2,
        "c0": _pack_c0(cs[0]),
        "c1": cs[1],
        "c2": cs[2],
        "c3": cs[3],
        "c4": cs[4],
    }
    in_maps = []
    ones_row = np.ones((1, R), dtype=np.float32)
    for c in range(NCORES):
        xs = x[c * R : (c + 1) * R].T.astype(np.float32)
        xs = np.ascontiguousarray(np.vstack([xs, ones_row]), dtype=np.float32)
        in_maps.append({"xT": xs, **shared})
    return in_maps


def kernel(x, B, W0, tW0, W1, tW1, W2, tW2, W3, tW3, W4, tW4):
    global LAST_RESULTS
    from concourse.bass_utils import run_bass_kernel_spmd

    if "nc" not in _CACHE:
        _CACHE["nc"] = _build()
    nc = _CACHE["nc"]

    in_maps = _prepare_inputs(
        np.asarray(x, dtype=np.float32),
        np.asarray(B, dtype=np.float32),
        [np.asarray(w, np.float32) for w in (W0, W1, W2, W3, W4)],
        [np.asarray(w, np.float32) for w in (tW0, tW1, tW2, tW3, tW4)],
    )
    res = run_bass_kernel_spmd(
        nc, in_maps, list(range(NCORES)), trace=TRACE, **TRACE_KWARGS
    )
    LAST_RESULTS = res
    out = np.concatenate([r["out"].reshape(-1) for r in res.results])
    return out.reshape(BATCH, 1).astype(np.float32)



# revision 15
# speedup vs baseline: 1.8538x; 1.8538x over previous
"""Trainium2 Bass kernel for nn_Cheby_KAN (FourierFeatures -> 5 Cheby-KAN layers -> cheby-gelu).

Self-contained: hardcodes shapes/sharding. Data-parallel over 8 NeuronCores
(batch 131072 -> 16384 rows/core).

v2 design (from v1 baseline at ~1.29 ms, tensor-bound 81%):
  - Layers 1-3 matmuls run in f32r (1 PE pass/512 cols vs fp32's 4).
    Probed semantics: f32r rounds BOTH operands round-to-nearest-even to
    11 mantissa bits; engine writes into f32r tiles round identically.
    Host emulation of the full pipeline puts the resulting output error
    at ~9.5e-3 (gate 2e-2, deterministic inputs). L0 + the Fourier proj
    stay fp32: their rounding is amplified ~60x downstream.
  - k=0 (constant) matmuls eliminated: layer biases are folded into the
    next layer's tanh activation bias (cumulative, since the residual
    chain lives in PSUM).
  - L4 (q=1) via even/odd split Horner on monomial coefficients: 2
    independent 4-step STT chains (DVE + GpSimd) + 2 ones-matmuls per
    chunk. No phi ladder needed for L4.
  - phi basis amplitude-normalized (Cheby-flavored) with per-slot
    ACT-Square vs DVE/GpSimd-STT forms, so pointwise work balances
    across all three engines (GpSimd was idle in v1).
  - G=1024 (2 PSUM banks per hps buffer) with 2 groups in flight and
    manual software pipelining: group g+1's Fourier+L0 ladder is
    emitted mid-group-g so no engine waits on the serial
    ladder->matmul->ladder chain.
"""

import math

import numpy as np

NCORES = 8
BATCH = 131072
R = BATCH // NCORES  # rows per core
G = 1024  # free-dim group size
F = 512  # matmul moving chunk (fp32 psum bank)
NG = R // G
NCH = G // F
DEG = 8

GELU_1 = 0.8413447460685429  # gelu(1), exact
S2 = float(np.float32(math.sqrt(2.0)))
S22 = float(np.float32(2.0 * math.sqrt(2.0)))
RC = 12582912.0  # 1.5 * 2**23 fp32 round-to-int constant

# mid layers (1..3) matmul dtype: f32r (True) or fp32 (False)
MID_F32R = [True, True, True]


def _pmul(a, b):
    return np.convolve(a, b)[:9]


def _phi_polys_l0():
    """L0 basis (fp32, v1's PHI2_ACT=True variant): power-coeff vectors."""
    x = np.zeros(9)
    x[1] = 1.0
    one = np.zeros(9)
    one[0] = 1.0
    p = [None] * 9
    p[0] = one
    p[1] = x
    p[2] = _pmul(S2 * x, S2 * x)  # 2x^2
    p[3] = _pmul(p[2] - one, p[1])
    p[4] = _pmul(S2 * p[2] - S2 * one, S2 * p[2] - S2 * one)  # (T4+1)
    p[5] = _pmul(p[4] - one, p[1])
    p[6] = _pmul(p[3], p[3])
    p[7] = _pmul(p[4] - one, p[3])
    p[8] = _pmul(S2 * p[4] - S2 * one, S2 * p[4] - S2 * one)  # (T8+1)
    return np.stack(p, 0)


def _phi_polys_mid_stt():
    """L1 basis (f32r ladder): amplitude-normalized, STT-x4 form.

    p2 = 2x^2 (ACT Square(S2 x));      p3 = (p2-1)*x       (STT)
    p4 = (p2-2)*p2 = (T4-1)/2 (STT);   p5 = (p4+1/2)*x     (STT)
    p6 = p3^2 (ACT Square);            p7 = (p4+1/2)*p3    (STT)
    p8 = Square(S22*p4 + S2) = T8+1 (ACT)
    """
    x = np.zeros(9)
    x[1] = 1.0
    one = np.zeros(9)
    one[0] = 1.0
    p = [None] * 9
    p[0] = one
    p[1] = x
    p[2] = _pmul(S2 * x, S2 * x)
    p[3] = _pmul(p[2] - one, p[1])
    p[4] = _pmul(p[2] - 2.0 * one, p[2])
    p[5] = _pmul(p[4] + 0.5 * one, p[1])
    p[6] = _pmul(p[3], p[3])
    p[7] = _pmul(p[4] + 0.5 * one, p[3])
    p[8] = _pmul(S22 * p[4] + S2 * one, S22 * p[4] + S2 * one)
    return np.stack(p, 0)


def _u_mono():
    M = np.zeros((9, 9))
    M[0, 0] = 1.0
    M[1, 1] = 2.0
    for n in range(2, 9):
        M[n, 1:] += 2.0 * M[n - 1, :-1]
        M[n, :] -= M[n - 2, :]
    return M


def _fold_layer(W, tW, phi):
    """C[p,k,q] (float32, contiguous) for basis `phi` (9x9 power coeffs)."""
    D = _u_mono() @ np.linalg.inv(phi)  # U_n = sum_k D[n,k] phi_k
    A = tW.astype(np.float64) * W.astype(np.float64)[:, :, None]  # [q,p,n]
    C = np.einsum("qpn,nk->pkq", A, D)
    return np.ascontiguousarray(C, dtype=np.float32)


def _pack_c0(C0):
    """[64,9,128] -> [128,4,128]: slots stack pairs (1,2),(3,4),(6,5),(7,8)."""
    c0p = np.zeros((128, 4, 128), dtype=np.float32)
    for j, (kt, kb) in enumerate([(1, 2), (3, 4), (6, 5), (7, 8)]):
        c0p[0:64, j, :] = C0[:, kt, :]
        c0p[64:128, j, :] = C0[:, kb, :]
    return np.ascontiguousarray(c0p)


def _r11(a):
    """round-to-nearest-even to 11 explicit mantissa bits (f32r grid)."""
    u = np.asarray(a, np.float32).view(np.uint32).astype(np.uint64)
    shift = 12
    half = np.uint64(1 << (shift - 1))
    low = u & np.uint64((1 << shift) - 1)
    base = u >> np.uint64(shift)
    up = (low > half) | ((low == half) & ((base & np.uint64(1)) == 1))
    out = (base + up.astype(np.uint64)) << np.uint64(shift)
    return out.astype(np.uint32).view(np.float32)


_CACHE = {}

TRACE = False
TRACE_KWARGS = {}
LAST_RESULTS = None


def _build():
    from concourse import bacc, bass, tile
    import concourse.mybir as mybir
    from concourse._compat import get_trn_type

    A = mybir.ActivationFunctionType
    OP = mybir.AluOpType
    f32 = mybir.dt.float32
    f32r = mybir.dt.float32r

    nc = bacc.Bacc(
        get_trn_type() or "TRN2",
        target_bir_lowering=False,
        debug=False,
        num_devices=NCORES,
    )

    # ---- DRAM I/O ----
    xT_d = nc.dram_tensor("xT", [9, R], f32, kind="ExternalInput").ap()
    b2_d = nc.dram_tensor("b2", [9, 128], f32, kind="ExternalInput").ap()
    c0_d = nc.dram_tensor("c0", [128, 4, 128], f32, kind="ExternalInput").ap()
    cm_d = [
        nc.dram_tensor(f"c{l}", [128, 8, 128], f32, kind="ExternalInput").ap()
        for l in (1, 2, 3)
    ]
    c4_d = nc.dram_tensor("c4", [128, 8], f32, kind="ExternalInput").ap()
    bv_d = nc.dram_tensor("biasv", [128, 5], f32, kind="ExternalInput").ap()
    h5_d = nc.dram_tensor("h5scratch", [R], f32, kind="Internal").ap()
    out_d = nc.dram_tensor("out", [128, R // 128], f32, kind="ExternalOutput").ap()

    with tile.TileContext(nc) as tc:
        with (
            tc.tile_pool(name="wpool", bufs=1) as wpool,
            tc.tile_pool(name="cpool", bufs=1) as cpool,
            tc.tile_pool(name="xpool", bufs=2) as xpool,
            tc.tile_pool(name="h0pool", bufs=2) as h0pool,
            tc.tile_pool(name="bpool", bufs=2) as bpool,
            tc.tile_pool(name="h5pool", bufs=2) as h5pool,
            tc.tile_pool(name="fpool", bufs=1) as fpool,
            tc.tile_pool(name="hps_pool", bufs=2, space=bass.MemorySpace.PSUM) as hpsp,
            tc.tile_pool(name="proj_pool", bufs=2, space=bass.MemorySpace.PSUM) as prjp,
            tc.tile_pool(name="l4_pool", bufs=2, space=bass.MemorySpace.PSUM) as l4p,
        ):
            # ---- constants / weights ----
            b2_sb = wpool.tile([9, 128], f32)
            nc.sync.dma_start(out=b2_sb[:], in_=b2_d)
            c0_sb = wpool.tile([128, 4, 128], f32)
            nc.sync.dma_start(out=c0_sb[:], in_=c0_d)
            cm_sb = []
            for i, cd in enumerate(cm_d):
                l = i + 1
                if MID_F32R[i]:
                    stage = wpool.tile([128, 8, 128], f32, tag="cstage")
                    nc.sync.dma_start(out=stage[:], in_=cd)
                    tr = wpool.tile([128, 8, 128], f32r, tag=f"c{l}r")
                    nc.vector.tensor_copy(tr[:], stage[:])
                    cm_sb.append(tr)
                else:
                    t32 = wpool.tile([128, 8, 128], f32, tag=f"c{l}")
                    nc.sync.dma_start(out=t32[:], in_=cd)
                    cm_sb.append(t32)
            c4_sb = wpool.tile([128, 8], f32)
            nc.sync.dma_start(out=c4_sb[:], in_=c4_d)
            bv_sb = wpool.tile([128, 5], f32)
            nc.sync.dma_start(out=bv_sb[:], in_=bv_d)

            bS2 = cpool.tile([128, 1], f32)
            nc.vector.memset(bS2[:], -S2)
            bpS2 = cpool.tile([128, 1], f32)
            nc.vector.memset(bpS2[:], S2)
            ones1 = cpool.tile([128, 1], f32)
            nc.vector.memset(ones1[:], 1.0)
            ones1r = cpool.tile([128, 1], f32r)
            nc.vector.tensor_copy(ones1r[:], ones1[:])

            mid_dt = [f32r if MID_F32R[i] else f32 for i in range(3)]

            st = {}  # per-group tile handles

            def stage_f(g):
                """Fourier + L0 ladder for group g."""
                s = {}
                h0 = h0pool.tile([128, G], f32, tag="h0")
                for c in range(NCH):
                    xt_g = xpool.tile([9, F], f32, tag="xg")
                    nc.sync.dma_start(
                        out=xt_g[:], in_=xT_d[:, g * G + c * F : g * G + (c + 1) * F]
                    )
                    pr = prjp.tile([128, F], f32, tag="proj")
                    nc.tensor.matmul(pr[:], b2_sb[:], xt_g[:], start=True, stop=True)
                    rr = xpool.tile([128, F], f32, tag="rr")
                    nc.vector.tensor_scalar(
                        rr[:], pr[:], RC, RC, op0=OP.add, op1=OP.subtract
                    )
                    nc.vector.tensor_tensor(rr[:], pr[:], rr[:], op=OP.subtract)
                    nc.scalar.activation(
                        h0[:, c * F : (c + 1) * F], rr[:], A.Sin, scale=2.0 * math.pi
                    )
                # L0 ladder (fp32, pair tiles for K=128 matmuls)
                xt = bpool.tile([128, G], f32, tag="pw1")
                nc.scalar.activation(xt[:], h0[:], A.Tanh)
                r12 = bpool.tile([128, G], f32, tag="r12")
                nc.scalar.activation(r12[0:64, :], h0[0:64, :], A.Tanh)
                nc.scalar.activation(r12[64:128, :], xt[64:128, :], A.Square, scale=S2)
                x2 = bpool.tile([128, G], f32, tag="pw2")
                nc.scalar.activation(x2[:], xt[:], A.Square, scale=S2)
                r34 = bpool.tile([128, G], f32, tag="r34")
                nc.vector.scalar_tensor_tensor(
                    r34[0:64, :], x2[0:64, :], -1.0, xt[0:64, :],
                    op0=OP.add, op1=OP.mult,
                )
                nc.scalar.activation(
                    r34[64:128, :], x2[64:128, :], A.Square,
                    bias=bS2[64:128, :], scale=S2,
                )
                x4 = bpool.tile([128, G], f32, tag="pw4")
                nc.scalar.activation(x4[:], x2[:], A.Square, bias=bS2[:], scale=S2)
                r56 = bpool.tile([128, G], f32, tag="r56")
                nc.scalar.activation(r56[0:64, :], r34[0:64, :], A.Square)
                nc.vector.scalar_tensor_tensor(
                    r56[64:128, :], x4[64:128, :], -1.0, xt[64:128, :],
                    op0=OP.add, op1=OP.mult,
                )
                r78 = bpool.tile([128, G], f32, tag="r78")
                nc.vector.scalar_tensor_tensor(
                    r78[0:64, :], x4[0:64, :], -1.0, r34[0:64, :],
                    op0=OP.add, op1=OP.mult,
                )
                nc.scalar.activation(
                    r78[64:128, :], x4[64:128, :], A.Square,
                    bias=bS2[64:128, :], scale=S2,
                )
                s["h0"] = h0
                s["pairs"] = (r12, r34, r56, r78)
                s["hps"] = hpsp.tile([128, G], f32, tag="hps", name="hps")
                st[g] = s

            def stage_m0(g):
                s = st[g]
                hps = s["hps"]
                for j, rt in enumerate(s["pairs"]):
                    for c in range(NCH):
                        cf = slice(c * F, (c + 1) * F)
                        nc.tensor.matmul(
                            hps[:, cf], c0_sb[:, j, :], rt[:, cf],
                            start=(j == 0), stop=False, skip_group_check=True,
                        )

            def stage_s(g, l):
                """Mid layer l in 1..3: tanh + f32r phi ladder.

                L1 uses the STT-x4 basis (one less ACT-write rounding in the
                x4 chain); L2/L3 use the ACT-x4 basis to balance engines.
                """
                s = st[g]
                dt = mid_dt[l - 1]
                hps = s["hps"]
                xt = bpool.tile([128, G], dt, tag="pw1")
                nc.scalar.activation(
                    xt[:], hps[:], A.Tanh, bias=bv_sb[:, l - 1 : l]
                )
                x2 = bpool.tile([128, G], dt, tag="pw2")
                nc.scalar.activation(x2[:], xt[:], A.Square, scale=S2)
                x3 = bpool.tile([128, G], dt, tag="pw3")
                nc.vector.scalar_tensor_tensor(
                    x3[:], x2[:], -1.0, xt[:], op0=OP.add, op1=OP.mult
                )
                x4 = bpool.tile([128, G], dt, tag="pw4")
                if l == 1:
                    nc.vector.scalar_tensor_tensor(
                        x4[:], x2[:], -2.0, x2[:], op0=OP.add, op1=OP.mult
                    )
                    s45 = 0.5
                else:
                    nc.scalar.activation(
                        x4[:], x2[:], A.Square, bias=bS2[:], scale=S2
                    )
                    s45 = -1.0
                x5 = bpool.tile([128, G], dt, tag="pw5")
                nc.vector.scalar_tensor_tensor(
                    x5[:], x4[:], s45, xt[:], op0=OP.add, op1=OP.mult
                )
                x6 = bpool.tile([128, G], dt, tag="pw6")
                nc.scalar.activation(x6[:], x3[:], A.Square)
                x7 = bpool.tile([128, G], dt, tag="pw7")
                nc.vector.scalar_tensor_tensor(
                    x7[:], x4[:], s45, x3[:], op0=OP.add, op1=OP.mult
                )
                x8 = bpool.tile([128, G], dt, tag="pw8")
                if l == 1:
                    nc.scalar.activation(
                        x8[:], x4[:], A.Square, bias=bpS2[:], scale=S22
                    )
                else:
                    nc.scalar.activation(
                        x8[:], x4[:], A.Square, bias=bS2[:], scale=S2
                    )
                s["x"] = [None, xt, x2, x3, x4, x5, x6, x7, x8]

            def stage_m(g, l):
                s = st[g]
                hps = s["hps"]
                x = s["x"]
                for k in range(1, 9):
                    for c in range(NCH):
                        cf = slice(c * F, (c + 1) * F)
                        nc.tensor.matmul(
                            hps[:, cf], cm_sb[l - 1][:, k - 1, :], x[k][:, cf],
                            start=False, stop=(l == 3 and k == 8),
                            skip_group_check=True,
                        )

            def stage_s4(g):
                """L4: tanh + straight 8-step Horner (coeffs of x^k in c4[:,k-1])."""
                s = st[g]
                hps = s["hps"]
                xt = bpool.tile([128, G], f32r, tag="pw1")
                nc.scalar.activation(xt[:], hps[:], A.Tanh, bias=bv_sb[:, 3:4])
                te = bpool.tile([128, G], f32r, tag="pw3")
                nc.vector.tensor_scalar(
                    te[:], xt[:], c4_sb[:, 7:8], 0.0, op0=OP.mult, op1=OP.add
                )
                for k in range(7, 0, -1):
                    nc.vector.scalar_tensor_tensor(
                        te[:], te[:], c4_sb[:, k - 1 : k], xt[:],
                        op0=OP.add, op1=OP.mult,
                    )
                s["te"] = te

            def stage_m4(g):
                s = st[g]
                for c in range(NCH):
                    cf = slice(c * F, (c + 1) * F)
                    l4 = l4p.tile([1, F], f32, tag="l4")
                    nc.tensor.matmul(
                        l4[:], ones1r[:], s["te"][:, cf],
                        start=True, stop=True, skip_group_check=True,
                    )
                    h5c = h5pool.tile([1, F], f32, tag="h5c")
                    nc.scalar.activation(h5c[:], l4[:], A.Copy)
                    off = g * G + c * F
                    nc.sync.dma_start(out=h5_d[off : off + F], in_=h5c[:])

            # ---- pipelined emission: 2 groups in flight ----
            stage_f(0)
            for g in range(NG):
                stage_m0(g)
                stage_s(g, 1)
                stage_m(g, 1)
                stage_s(g, 2)
                stage_m(g, 2)
                if g + 1 < NG:
                    stage_f(g + 1)
                stage_s(g, 3)
                stage_m(g, 3)
                stage_s4(g)
                stage_m4(g)
                del st[g]

            # ---- final stage: cheby-gelu (standard T_n, deg 5) ----
            W = R // 128
            h5r = fpool.tile([128, W], f32, tag="h5r")
            nc.sync.dma_start(out=h5r[:], in_=h5_d.rearrange("(a b) -> a b", a=128))
            xt = fpool.tile([128, W], f32, tag="fxt")
            # b0 (L4 constant term) folded in here: tanh(h5 + b0)
            nc.scalar.activation(xt[:], h5r[:], A.Tanh, bias=bv_sb[:, 4:5])
            s = fpool.tile([128, W], f32, tag="fs")
            nc.scalar.activation(s[:], xt[:], A.Square)
            t2 = fpool.tile([128, W], f32, tag="ft2")
            nc.vector.tensor_scalar(t2[:], s[:], 2.0, -1.0, op0=OP.mult, op1=OP.add)
            t3 = fpool.tile([128, W], f32, tag="ft3")
            m = fpool.tile([128, W], f32, tag="fm")
            nc.vector.scalar_tensor_tensor(m[:], xt[:], 2.0, t2[:], op0=OP.mult, op1=OP.mult)
            nc.vector.tensor_tensor(t3[:], m[:], xt[:], op=OP.subtract)
            t4 = fpool.tile([128, W], f32, tag="ft4")
            m = fpool.tile([128, W], f32, tag="fm")
            nc.vector.scalar_tensor_tensor(m[:], xt[:], 2.0, t3[:], op0=OP.mult, op1=OP.mult)
            nc.vector.tensor_tensor(t4[:], m[:], t2[:], op=OP.subtract)
            t5 = fpool.tile([128, W], f32, tag="ft5")
            m = fpool.tile([128, W], f32, tag="fm")
            nc.vector.scalar_tensor_tensor(m[:], xt[:], 2.0, t4[:], op0=OP.mult, op1=OP.mult)
            nc.vector.tensor_tensor(t5[:], m[:], t3[:], op=OP.subtract)
            gsum = fpool.tile([128, W], f32, tag="fgsum")
            gt = fpool.tile([128, W], f32, tag="fgt")
            nc.scalar.activation(gsum[:], xt[:], A.Gelu)  # gelu(T1)
            for tt in (t2, t3, t4):
                nc.scalar.activation(gt[:], tt[:], A.Gelu)
                nc.vector.tensor_tensor(gsum[:], gsum[:], gt[:], op=OP.add)
            nc.scalar.activation(gt[:], t5[:], A.Gelu)
            nc.vector.scalar_tensor_tensor(
                s[:], gt[:], GELU_1, gsum[:], op0=OP.add, op1=OP.add
            )
            nc.sync.dma_start(out=out_d, in_=s[:])

    nc.compile()
    return nc


def _prepare_inputs(x, B, Ws, tWs):
    # turns units; 4 stacked copies of B (128 partitions of duplicated m+shift)
    b1 = np.concatenate([B, B, B, B], axis=1).astype(np.float32)  # [8, 128]
    qshift = np.concatenate([np.zeros(32), np.full(32, 0.25)])
    shift_row = np.concatenate([qshift, qshift]).astype(np.float32)  # [128]
    b2 = np.ascontiguousarray(np.vstack([b1, shift_row[None, :]]), dtype=np.float32)

    phi0 = _phi_polys_l0()
    phi_stt = _phi_polys_mid_stt()
    # L1: STT-x4 basis; L2/L3: ACT-x4 basis (same polys as L0)
    Cm = [
        _fold_layer(Ws[1], tWs[1], phi_stt),
        _fold_layer(Ws[2], tWs[2], phi0),
        _fold_layer(Ws[3], tWs[3], phi0),
    ]
    C0 = _fold_layer(Ws[0], tWs[0], phi0)

    # cumulative per-partition biases (k=0 terms), folded into tanh biases
    biases = [C0[:, 0, :].astype(np.float64).sum(axis=0)]
    for Cl in Cm:
        biases.append(Cl[:, 0, :].astype(np.float64).sum(axis=0))
    cb = np.cumsum(np.stack(biases), axis=0)  # [4,128]

    # L4 monomial coefficients
    U = _u_mono()
    A4 = tWs[4].astype(np.float64)[0] * Ws[4].astype(np.float64)[0][:, None]  # [128,9]
    mono = A4 @ U  # [128, 9] power coeffs
    b0 = mono[:, 0].sum()
    c4 = np.ascontiguousarray(mono[:, 1:], dtype=np.float32)  # [128,8] x^1..x^8

    biasv = np.zeros((128, 5), dtype=np.float32)
    biasv[:, 0:4] = cb.T.astype(np.float32)
    biasv[:, 4] = np.float32(b0)

    shared = {
        "biasv": biasv,
        "b2": b2,
        "c0": _pack_c0(C0),
        "c4": c4,
    }
    for i, Cl in enumerate(Cm):
        arr = np.ascontiguousarray(Cl[:, 1:9, :])
        if MID_F32R[i]:
            arr = _r11(arr)
        shared[f"c{i + 1}"] = arr

    in_maps = []
    ones_row = np.ones((1, R), dtype=np.float32)
    for c in range(NCORES):
        xs = x[c * R : (c + 1) * R].T.astype(np.float32)
        xs = np.ascontiguousarray(np.vstack([xs, ones_row]), dtype=np.float32)
        in_maps.append({"xT": xs, **shared})
    return in_maps


def kernel(x, B, W0, tW0, W1, tW1, W2, tW2, W3, tW3, W4, tW4):
    global LAST_RESULTS
    from concourse.bass_utils import run_bass_kernel_spmd

    if "nc" not in _CACHE:
        _CACHE["nc"] = _build()
    nc = _CACHE["nc"]

    in_maps = _prepare_inputs(
        np.asarray(x, dtype=np.float32),
        np.asarray(B, dtype=np.float32),
        [np.asarray(w, np.float32) for w in (W0, W1, W2, W3, W4)],
        [np.asarray(w, np.float32) for w in (tW0, tW1, tW2, tW3, tW4)],
    )
    res = run_bass_kernel_spmd(
        nc, in_maps, list(range(NCORES)), trace=TRACE, **TRACE_KWARGS
    )
    LAST_RESULTS = res
    out = np.concatenate([r["out"].reshape(-1) for r in res.results])
    return out.reshape(BATCH, 1).astype(np.float32)
